# revision 22
# baseline (speedup 1.0000x reference)
"""Trainium2 Bass kernel for nn_CustomTransformer_60619168416497.

kernel(**inputs) takes the FULL unsharded inputs (as produced by
setup_inputs()) and returns the FULL output (scalar f32 loss), running the
heavy X-dependent work on 8 NeuronCores (data parallel over the batch).

-- Algebraic reduction -------------------------------------------------------
Only h_2[:, -1] (the cls row) reaches the output head, so the attention never
needs materializing. Folding the tiny weights on the host:
    w     = W1 @ W_k @ (cls@W_q) / sqrt(32)     [8]
    N     = W1 @ W_v @ W2                       [8,2]
    a_cls = cls . (W_k @ (cls@W_q))/sqrt(32)    scalar
Per batch b (normalized x = (X - mu)/sigma'), the 257-way softmax needs only
    M    = max_j alpha*t_j          (t_j = X[b,j,:] @ w)
    den  = sum_j exp(alpha*t_j - M)
    G2_c = sum_j exp(alpha*t_j - M) * (X[b,j,:] @ N[:,c])
from which the host recovers z[b] and the NLL in closed form (f64).  mu and
sigma are global scalars over all of X; the host computes them exactly in f64
during input prep (prep already touches every element for the transpose/cast),
so a single device launch suffices.

-- Device work (per core, 256 batches, ONE launch) ---------------------------
Packed layout: planes P[i*16+u, v*256+j] = bf16(alpha*w_i * X[b,j,i]) with
local batch b = u*16+v.  A single stationary [128,48] computes all three
per-token contractions in ONE PE pass over the 4096 columns (each X element
streams through the PE exactly once):
    psum[q*16+u, v*256+j],  q=0: alpha*t (coeff 1), q=1/2: r_c (coeff N_c/aw)
A PSUM->SBUF copy then an SBUF->SBUF "bridge" DMA regroups partitions
(u, col-chunk g) -> partition u*8+g so softmax post-ops (max / Exp+accum /
mul+reduce) run at full 128-lane occupancy on [128, 512] tiles.  Out: per
batch M, den, G2 -> [128, 8] f32; host finishes the loss in f64.
The NEFF is input-independent, so compilation caches across calls.
"""
import numpy as np
import ml_dtypes

import concourse.tile as tile
import concourse.mybir as mybir
from concourse import bacc
from concourse.bass_utils import run_bass_kernel_spmd

F32 = mybir.dt.float32
BF16 = mybir.dt.bfloat16
NCORES = 8
BPC = 256          # batches per core
L = 256            # tokens
I = 8              # features
H = 32
EPS = 1e-7
PCOLS = 4096       # v*256 + j

bf16 = ml_dtypes.bfloat16


# ---------------------------------------------------------------- host math
def _fold_weights(W1, cls_tok, W_q, W_k, W_v, W_t, W2):
    f8 = np.float64
    W1, cls_tok, W_q, W_k, W_v, W_t, W2 = [np.asarray(a, f8) for a in
                                           (W1, cls_tok, W_q, W_k, W_v, W_t, W2)]
    u = (W_k @ (cls_tok @ W_q)) / np.sqrt(f8(H))
    w = W1 @ u
    N = (W1 @ W_v) @ W2
    return dict(
        w=w, N=N,
        a_cls=float(cls_tok @ u),
        sumw=float(w.sum()),
        n1=N.sum(axis=0),
        v2=(cls_tok @ W_v) @ W2,
        t2=(cls_tok @ W_t) @ W2,
    )


def _host_stats(X):
    Xd = np.asarray(X, np.float64)
    mu = Xd.mean()
    sigma = Xd.std(ddof=1) + EPS
    return float(mu), float(sigma), float(1.0 / sigma)


def _prep_inputs(X, coef, Wst):
    """Per-core packed input [128, 48 + 4096]: stationary cols then planes
    (i,u) x (v,j), planes scaled by coef[i]."""
    X = np.asarray(X, np.float32)
    per_core = []
    for c in range(NCORES):
        xc = X[c * BPC:(c + 1) * BPC].reshape(16, 16, L, I)     # [u, v, j, i]
        a = (xc * coef[None, None, None, :]).astype(bf16)
        planes = np.ascontiguousarray(a.transpose(3, 0, 1, 2)).reshape(128, PCOLS)
        per_core.append(np.concatenate([Wst, planes], axis=1))
    return per_core


def _build_stationary(c0, c1, c2):
    """Wst[i*16+u, q*16+u] = cq[i]  (q=0: t, q=1: r0, q=2: r1)."""
    Wst = np.zeros((128, 48), np.float32)
    iu = np.arange(128)
    i_idx, u_idx = iu // 16, iu % 16
    Wst[iu, u_idx] = c0[i_idx]
    Wst[iu, 16 + u_idx] = c1[i_idx]
    Wst[iu, 32 + u_idx] = c2[i_idx]
    return Wst.astype(bf16)


# ---------------------------------------------------------------- device body
def _main_body(nc):
    xp = nc.dram_tensor("xp", [128, 48 + PCOLS], BF16, kind="ExternalInput")
    outd = nc.dram_tensor("out", [128, 8], F32, kind="ExternalOutput")

    with tile.TileContext(nc) as tc:
        with (
            tc.tile_pool(name="xpool", bufs=1) as xpool,
            tc.tile_pool(name="ps", bufs=1, space="PSUM") as ps,
            tc.tile_pool(name="work", bufs=1) as work,
            tc.tile_pool(name="outp", bufs=1) as outp,
        ):
            # chunk0 carries the stationary (cols 0:48) + first 2 matmul blocks
            xt = [xpool.tile([128, 1072 if k == 0 else 1024], BF16,
                             name=f"x{k}", tag=f"x{k}") for k in range(4)]
            for k in range(4):
                eng = nc.sync if k % 2 == 0 else nc.scalar
                lo = 0 if k == 0 else 48 + k * 1024
                eng.dma_start(xt[k][:], xp[:, lo:48 + (k + 1) * 1024])
            wt = xt[0][:, 0:48]

            # one PSUM tile spanning all 8 banks; 8 single-pass matmuls
            pt = ps.tile([48, PCOLS], F32, name="pt", tag="pt")
            for k in range(8):
                off = 48 if k // 2 == 0 else 0
                nc.tensor.matmul(pt[:, k * 512:(k + 1) * 512], wt,
                                 xt[k // 2][:, off + (k % 2) * 512:
                                            off + (k % 2) * 512 + 512],
                                 start=True, stop=True, skip_group_check=True)

            # PSUM -> SBUF staging (bf16), chunk-wise on ACT/DVE/Pool
            st = work.tile([48, PCOLS], BF16, name="st", tag="st")
            for k in range(8):
                sl = slice(k * 512, (k + 1) * 512)
                if k % 2 == 0:
                    nc.scalar.copy(st[:, sl], pt[:, sl])
                else:
                    nc.vector.tensor_copy(st[:, sl], pt[:, sl])

            # bridge: [48, 4096] -> t3 [128, 1536]  (t | r0 | r1 slots)
            # dst partition u*8+g <- src (row q*16+u, col-chunk g)
            t3 = work.tile([128, 1536], BF16, name="t3", tag="t3")
            engs = (nc.sync, nc.scalar, nc.sync)
            for q in range(3):
                engs[q].dma_start(
                    t3[:, q * 512:(q + 1) * 512],
                    st[q * 16:(q + 1) * 16, :].rearrange("u (g c) -> u g c", g=8))

            out = outp.tile([128, 8], F32, name="out", tag="out")
            negaM = work.tile([128, 2], F32, name="negaM", tag="negaM")
            e = work.tile([128, 512], BF16, name="e", tag="e")

            # negaM = -max_j t  (fused negate); host recovers M = -out[:,0:2]
            nc.vector.tensor_reduce(
                negaM[:], t3[:, 0:512].rearrange("p (b j) -> p b j", b=2),
                axis=mybir.AxisListType.X, op=mybir.AluOpType.max, negate=True)
            nc.vector.tensor_copy(out[:, 0:2], negaM[:])
            for h in range(2):
                sl = slice(h * 256, (h + 1) * 256)
                nc.scalar.activation(e[:, sl], t3[:, sl],
                                     mybir.ActivationFunctionType.Exp,
                                     bias=negaM[:, h:h + 1],
                                     accum_out=out[:, 2 + h:3 + h])

            scr = work.tile([128, 1024], BF16, name="scr", tag="scr")
            nc.vector.tensor_mul(scr[:, 0:512], e[:], t3[:, 512:1024])
            nc.gpsimd.tensor_mul(scr[:, 512:1024], e[:], t3[:, 1024:1536])
            nc.vector.tensor_reduce(
                out[:, 4:8], scr[:].rearrange("p (s j) -> p s j", s=4),
                axis=mybir.AxisListType.X, op=mybir.AluOpType.add)
            nc.sync.dma_start(outd[:], out[:])
    return nc


# ---------------------------------------------------------------- host finish
def _host_finish(outs, fold, mu, alpha, y):
    O = np.stack([np.asarray(o, np.float64) for o in outs])   # [8, 128, 8]
    # batch order: (core, u, g, half) = core*256 + u*16 + 2g + half
    A = O.reshape(NCORES, 16, 8, 8)
    M = -A[..., 0:2].reshape(-1)          # device ships negaM
    den = A[..., 2:4].reshape(-1)
    G2 = np.stack([A[..., 4:6].reshape(-1), A[..., 6:8].reshape(-1)], axis=1)
    a_cls, sumw, n1, v2, t2 = (fold["a_cls"], fold["sumw"], fold["n1"],
                               fold["v2"], fold["t2"])
    l_shift = M - alpha * mu * sumw
    m_full = np.maximum(l_shift, a_cls)
    scale_tok = np.exp(l_shift - m_full)
    e_cls = np.exp(a_cls - m_full)
    denom = den * scale_tok + e_cls
    S_cls = e_cls / denom
    gN = G2 * scale_tok[:, None] / denom[:, None]
    z = (gN - (mu * (1.0 - S_cls))[:, None] * n1[None, :]) * alpha \
        + S_cls[:, None] * v2[None, :] + t2[None, :]
    zmax = z.max(axis=1)
    lse = zmax + np.log(np.exp(z[:, 0] - zmax) + np.exp(z[:, 1] - zmax))
    y = np.asarray(y).astype(np.int64).reshape(-1)
    zy = np.take_along_axis(z, y[:, None], axis=1)[:, 0]
    return (lse - zy).mean()


# ---------------------------------------------------------------- entry point
_NC_CACHE = {}


def _get_nc():
    if "main" not in _NC_CACHE:
        nc = bacc.Bacc("TRN2", target_bir_lowering=False, debug=False,
                       num_devices=NCORES)
        _main_body(nc)
        nc.compile()
        _NC_CACHE["main"] = nc
    return _NC_CACHE["main"]


def kernel(X, y, W1, cls_tok, W_q, W_k, W_v, W_t, W2):
    fold = _fold_weights(W1, cls_tok, W_q, W_k, W_v, W_t, W2)
    mu, sigma, alpha = _host_stats(X)
    w, N = fold["w"], fold["N"]
    aw = alpha * w
    if np.abs(w).min() >= 1e-3 * max(np.abs(w).max(), 1.0):
        # pre-scaled planes: ONE bf16 rounding on the exp-sensitive t path
        Wst = _build_stationary(np.ones(I, np.float64), N[:, 0] / aw, N[:, 1] / aw)
        per_core = _prep_inputs(X, aw.astype(np.float64), Wst)
    else:
        # near-zero w entry: raw planes, coefficients in the stationary
        Wst = _build_stationary(aw, N[:, 0], N[:, 1])
        per_core = _prep_inputs(X, np.ones(I, np.float64), Wst)

    nc = _get_nc()
    ins = [{"xp": p} for p in per_core]
    res = run_bass_kernel_spmd(nc, ins, core_ids=list(range(NCORES)))
    loss = _host_finish([r["out"] for r in res.results], fold, mu, alpha, y)
    return np.float32(loss)



# revision 27
# speedup vs baseline: 1.3492x; 1.3492x over previous
"""Trainium2 Bass kernel for nn_CustomTransformer_60619168416497.

kernel(**inputs) takes the FULL unsharded inputs (as produced by
setup_inputs()) and returns the FULL output (scalar f32 loss), running the
heavy X-dependent work on 8 NeuronCores (data parallel over the batch).

-- Algebraic reduction -------------------------------------------------------
Only h_2[:, -1] (the cls row) reaches the output head, and with 2 classes only
the logit DIFFERENCE d = z0 - z1 is needed (nll = softplus(+-d)).  Folding the
tiny weights on the host:
    aw = alpha * W1 @ W_k @ (cls@W_q) / sqrt(32)   [8]   (alpha = 1/(std+eps))
    dG = W1 @ W_v @ (W2[:,0]-W2[:,1])              [8]
Per batch b the 257-way softmax needs only (t_j = X[b,j,:]@aw, r_j = X[b,j,:]@dG)
    M = max_j t_j,  den = sum_j exp(t_j-M),  S2 = sum_j exp(t_j-M)*r_j
from which the host recovers d and the NLL in closed form (f64; cls token and
global-mean corrections are scalar constants).  mu/sigma are computed on host
during input prep (which already touches every element for the fp8 packing).

-- Device work (per core, 256 batches, ONE launch) ---------------------------
fp8(e4m3) planes P[(i*16+u2), g*512+pr*256+j] = fp8(X[b,j,i]), b = g*32+u2*2+pr.
One DoubleRow matmul pair per group g (fp8 stationary split hi/lo and
accumulated in PSUM to recover bf16-level coefficient precision) computes both
contractions for 32 batches at once:  psum[q*32+u, g*256+j], q=0: t, q=1: r.
PSUM -> SBUF bf16 staging (ACT/DVE), then per half (4 groups = 128 batches) a
single rearranging DMA gives one-batch-per-partition [128, 256] tiles; softmax
post-ops are DVE max (negated), ACT Exp(bias=-M, accum_out=den) and a fused
DVE tensor_tensor_reduce for S2.  Input halves arrive via a prepared SWDGE
gather (fires before the HWDGE path could) + a plain DMA; the [128, 6] result
leaves via a prepared dma_scatter_add into a pre-zeroed DRAM buffer, cutting
the HWDGE+DGE latency out of the tail.  Host finishes the loss in f64.
The NEFF is input-independent, so compilation caches across calls.
"""
import numpy as np
import ml_dtypes

import concourse.tile as tile
import concourse.mybir as mybir
from concourse import bacc
from concourse.bass_utils import run_bass_kernel_spmd

F32 = mybir.dt.float32
BF16 = mybir.dt.bfloat16
FP8 = mybir.dt.float8e4
I16 = mybir.dt.int16
NCORES = 8
BPC = 256          # batches per core
L = 256            # tokens
I = 8              # features
H = 32
EPS = 1e-7
PCOLS = 4096       # g*512 + pr*256 + j

f8 = ml_dtypes.float8_e4m3
bf16 = ml_dtypes.bfloat16

# bridge DMA engine assignment (tA, rA, tB, rB): s=SP, a=ACT, g=Pool/SWDGE
_BRIDGE_ENGS = __import__("os").environ.get("BRIDGE_ENGS", "sgsg")
# prepared-scatter output path (faster tail, exercises SWDGE prep/trigger)
_USE_SCATTER = __import__("os").environ.get("OUT_SCATTER", "1") == "1"
_PE_WARM = __import__("os").environ.get("PE_WARM", "1") == "1"
_PLANES_FP8 = __import__("os").environ.get("PLANES", "bf16") == "fp8"
_USE_TTR = __import__("os").environ.get("USE_TTR", "0") == "1"
_USE_DR = __import__("os").environ.get("USE_DR", "0") == "1"
PDT_NP = f8 if _PLANES_FP8 else bf16


# ---------------------------------------------------------------- host math
def _fold_weights(X, W1, cls_tok, W_q, W_k, W_v, W_t, W2):
    fd = np.float64
    W1, cls_tok, W_q, W_k, W_v, W_t, W2 = [np.asarray(a, fd) for a in
                                           (W1, cls_tok, W_q, W_k, W_v, W_t, W2)]
    Xd = np.asarray(X, fd)
    mu = Xd.mean()
    alpha = 1.0 / (Xd.std(ddof=1) + EPS)
    wv = W_k @ (cls_tok @ W_q) / np.sqrt(fd(H))
    w = W1 @ wv
    G = (W1 @ W_v) @ W2
    v2 = (cls_tok @ W_v) @ W2
    t2 = (cls_tok @ W_t) @ W2
    return dict(
        aw=alpha * w, dG=G[:, 0] - G[:, 1],
        a_cls=float(cls_tok @ wv),
        sumw=float(w.sum()), dn1=float((G[:, 0] - G[:, 1]).sum()),
        dv2=float(v2[0] - v2[1]), dt2=float(t2[0] - t2[1]),
        mu=float(mu), alpha=float(alpha),
    )


def _hi_lo(v):
    hi = np.asarray(v, f8).astype(np.float64)
    lo = np.asarray(v - hi, f8).astype(np.float64)
    return hi, lo


def _build_stationary(aw, dG):
    """st[128, 256]: cols hl*128 + pr*64 + q*32 + (u2*2+pr) = c[hl][q][i]."""
    st = np.zeros((128, 256), np.float64)
    iu = np.arange(128)
    i_idx, u2 = iu // 16, iu % 16
    if _PLANES_FP8:
        aw_hi, aw_lo = _hi_lo(aw)
        dg_hi, dg_lo = _hi_lo(dG)
    else:
        aw_hi, aw_lo = aw, np.zeros_like(aw)
        dg_hi, dg_lo = dG, np.zeros_like(dG)
    cs = {(0, 0): aw_hi, (0, 1): dg_hi, (1, 0): aw_lo, (1, 1): dg_lo}
    for hl in (0, 1):
        for q in (0, 1):
            for pr in (0, 1):
                st[iu, hl * 128 + pr * 64 + q * 32 + u2 * 2 + pr] = cs[hl, q][i_idx]
    return st.astype(PDT_NP)


def _prep_planes(X):
    """Per-core [128, 4096] fp8 planes: [(i,u2), (g,pr,j)]."""
    Xf = np.asarray(X, np.float32)
    per_core = []
    for c in range(NCORES):
        xc = Xf[c * BPC:(c + 1) * BPC].reshape(8, 16, 2, L, I)   # [g,u2,pr,j,i]
        pl = np.ascontiguousarray(xc.transpose(4, 1, 0, 2, 3)).reshape(128, PCOLS)
        per_core.append(pl.astype(PDT_NP))
    return per_core


# ---------------------------------------------------------------- device body
def _main_body(nc):
    PDT = FP8 if _PLANES_FP8 else BF16
    xp1 = nc.dram_tensor("xp1", [128, 2304], PDT, kind="ExternalInput")
    xp2 = nc.dram_tensor("xp2", [128, 2048], PDT, kind="ExternalInput")
    xi = nc.dram_tensor("xi", [128, 8], I16, kind="ExternalInput")
    outd = nc.dram_tensor("out", [128, 64], F32, kind="ExternalOutput")

    ssem = nc.alloc_semaphore("scatter_dma")
    with tile.TileContext(nc) as tc:
        with (
            tc.tile_pool(name="xpool", bufs=1) as xpool,
            tc.tile_pool(name="ps", bufs=1, space="PSUM") as ps,
            tc.tile_pool(name="work", bufs=1) as work,
        ):
            pl1 = xpool.tile([128, 2304], PDT, name="pl1", tag="pl1")
            pl2 = xpool.tile([128, 2048], PDT, name="pl2", tag="pl2")
            out = work.tile([128, 64], F32, name="out", tag="out")
            zero = work.tile([128, 64], F32, name="zero", tag="zero")
            idx = work.tile([128, 8], I16, name="idx", tag="idx")
            sb = [work.tile([64, 1024], BF16, name=f"sb{h}", tag=f"sb{h}")
                  for h in range(2)]
            t3 = [work.tile([128, 256], BF16, name=f"t3{h}", tag=f"t3{h}")
                  for h in range(2)]
            r3 = [work.tile([128, 256], BF16, name=f"r3{h}", tag=f"r3{h}")
                  for h in range(2)]
            eb = [work.tile([128, 512], BF16, name=f"eb{h}", tag=f"eb{h}")
                  for h in range(2)]

            # -- input loads (planes for groups 0-3 + stationary, then 4-7)
            nc.sync.dma_start(pl1[:], xp1[:])
            nc.scalar.dma_start(pl2[:], xp2[:])

            # -- prologue: zero the output DRAM (scatter-add needs a clean
            # base), prep the output scatter descriptors early so the final
            # trigger skips the HWDGE+DGE latency
            if _USE_SCATTER:
                nc.sync.dma_start(idx[:], xi[:])
                nc.vector.memset(zero[:], 0.0)
                nc.vector.memset(out[:], 0.0)
                nc.scalar.dma_start(outd[:], zero[:])
                nc.gpsimd.dma_scatter_add(
                    outd[:], out[:].rearrange("p (s e) -> p s e", s=1), idx[:],
                    128, 128, 64, prepare_only=True, sem=ssem, queue_num=0)

            # -- per half: 8 DoubleRow matmuls (hi/lo accumulated, one psum
            # tile per bank for fine-grained deps), staging, bridge, softmax
            st_hi = pl1[:, 2048:2176].rearrange("p (pr m) -> p pr m", pr=2)
            st_lo = pl1[:, 2176:2304].rearrange("p (pr m) -> p pr m", pr=2)
            pt = [ps.tile([64, 512], F32, name=f"pt{b}", tag=f"pt{b}")
                  for b in range(4)]
            psem = nc.alloc_semaphore("pe_warm")
            for h in range(2):
                for b in (2 * h, 2 * h + 1):
                    for g in (2 * b, 2 * b + 1):
                        pl = pl1 if g < 4 else pl2
                        lo = (g % 4) * 512
                        rhs = pl[:, lo:lo + 512].rearrange("p (pr n) -> p pr n",
                                                           pr=2)
                        o = pt[b][:, (g % 2) * 256:(g % 2) * 256 + 256]
                        mm = nc.tensor.matmul(
                            o, st_hi, rhs, start=(g % 2 == 0), stop=False,
                            perf_mode=mybir.MatmulPerfMode.DoubleRow,
                            skip_group_check=True)
                        if g == 0 and _PE_WARM:
                            # stall PE SEQ past the p-state warmup window so
                            # later matmuls dispatch at mid speed
                            nc.tensor.drain()
                        nc.tensor.matmul(o, st_lo, rhs,
                                         start=False, stop=(g % 2 == 1),
                                         perf_mode=mybir.MatmulPerfMode.DoubleRow,
                                         skip_group_check=True)
                    # stage bank -> sb[h] (ACT for half 0, DVE for half 1)
                    sl = slice((b % 2) * 512, (b % 2) * 512 + 512)
                    if h == 0:
                        nc.scalar.copy(sb[h][:, sl], pt[b][:])
                    else:
                        nc.vector.tensor_copy(sb[h][:, sl], pt[b][:])
                # bridge: [32, 1024] -> [128, 256] (batch-per-partition);
                # engine split avoids SEQ head-of-line blocking across halves
                engs = {"s": nc.sync, "a": nc.scalar, "g": nc.gpsimd}
                code = _BRIDGE_ENGS
                engs[code[2 * h]].dma_start(
                    t3[h][:], sb[h][0:32, :].rearrange("u (g j) -> u g j", g=4))
                engs[code[2 * h + 1]].dma_start(
                    r3[h][:], sb[h][32:64, :].rearrange("u (g j) -> u g j", g=4))

            # -- softmax partials per half: cols 3h+0: -M, 3h+1: den, 3h+2: S2
            for h in range(2):
                negaM = out[:, 3 * h:3 * h + 1]
                nc.vector.tensor_reduce(negaM, t3[h][:],
                                        axis=mybir.AxisListType.X,
                                        op=mybir.AluOpType.max, negate=True)
                nc.scalar.activation(eb[h][:, 0:256], t3[h][:],
                                     mybir.ActivationFunctionType.Exp,
                                     bias=negaM,
                                     accum_out=out[:, 3 * h + 1:3 * h + 2])
                if _USE_TTR:
                    nc.vector.tensor_tensor_reduce(
                        out=eb[h][:, 256:512], in0=eb[h][:, 0:256], in1=r3[h][:],
                        scale=1.0, scalar=0.0,
                        op0=mybir.AluOpType.mult, op1=mybir.AluOpType.add,
                        accum_out=out[:, 3 * h + 2:3 * h + 3])
                else:
                    nc.vector.tensor_mul(eb[h][:, 256:512], eb[h][:, 0:256],
                                         r3[h][:])
                    nc.vector.tensor_reduce(out[:, 3 * h + 2:3 * h + 3],
                                            eb[h][:, 256:512],
                                            axis=mybir.AxisListType.X,
                                            op=mybir.AluOpType.add)

            # -- epilogue: fire the prepared scatter (deferred reads of `out`
            # make it wait for all six result columns)
            if _USE_SCATTER:
                nc.gpsimd.trigger_dma(count=None, queue_num=0)
            else:
                nc.sync.dma_start(outd[:, 0:8], out[:, 0:8])

    if not _USE_SCATTER:
        return nc
    # Post-schedule fixup: Tile assigned the scatter prep to a DMASW proc
    # lane and generated downstream waits on that lane's semaphore, but the
    # DMA-completion sem baked into the descriptor is the user-provided
    # `ssem`.  Rewrite the prep's completion update (and sem_num field if
    # present) to the lane semaphore so the completion actually satisfies
    # the generated waits (sim and hardware read the same BIR).
    from concourse.tile_scheduler import PROC_NAMES
    prep = None
    waits_by_name = {}
    for blk in nc.m.functions[0].blocks:
        for ins in blk.instructions:
            if type(ins).__name__ == "InstDMAScatterAddAnt":
                prep = ins
            if ins.sync_info is not None:
                for w in ins.sync_info.on_wait:
                    if w.ant_name:
                        waits_by_name[w.ant_name] = w
    assert prep is not None
    lane = PROC_NAMES[prep.bass_scheduled_proc]
    assert lane.startswith("DMASW"), lane
    lane_waits = [w for n, w in waits_by_name.items()
                  if n.startswith(lane + "_")]
    assert lane_waits, f"no waits on {lane} found"
    si = prep.sync_info
    ups = list(si.on_update)
    patched = False
    for u in ups:
        if u.ant_name == "scatter_dma":
            u.id = lane_waits[0].id
            u.ant_name = lane_waits[0].ant_name
            patched = True
    assert patched
    si.on_update = ups
    prep.sync_info = si
    if hasattr(prep, "sem_num"):
        prep.sem_num = lane_waits[0].id
    return nc


# ---------------------------------------------------------------- host finish
def _host_finish(outs, fold, y):
    O = np.stack([np.asarray(o, np.float64) for o in outs])   # [8, 128, 8]
    p = np.arange(128)
    M = np.empty(NCORES * BPC); den = np.empty(NCORES * BPC); S2 = np.empty(NCORES * BPC)
    for h in range(2):
        b_loc = (h * 4 + (p & 3)) * 32 + (p >> 2)             # [128]
        for c in range(NCORES):
            bg = c * BPC + b_loc
            M[bg] = -O[c, :, 3 * h + 0]
            den[bg] = O[c, :, 3 * h + 1]
            S2[bg] = O[c, :, 3 * h + 2]
    alpha, mu = fold["alpha"], fold["mu"]
    l_shift = M - alpha * mu * fold["sumw"]
    m_full = np.maximum(l_shift, fold["a_cls"])
    scale_tok = np.exp(l_shift - m_full)
    e_cls = np.exp(fold["a_cls"] - m_full)
    denom = den * scale_tok + e_cls
    S_cls = e_cls / denom
    gsum = S2 * scale_tok / denom
    d = (gsum - mu * (1.0 - S_cls) * fold["dn1"]) * alpha \
        + S_cls * fold["dv2"] + fold["dt2"]
    y = np.asarray(y).astype(np.int64).reshape(-1)
    s = np.where(y == 0, -d, d)
    return np.logaddexp(0.0, s).mean()


# ---------------------------------------------------------------- entry point
_NC_CACHE = {}


def _get_nc():
    if "main" not in _NC_CACHE:
        nc = bacc.Bacc("TRN2", target_bir_lowering=False, debug=False,
                       num_devices=NCORES)
        _main_body(nc)
        nc.compile()
        _NC_CACHE["main"] = nc
    return _NC_CACHE["main"]


def kernel(X, y, W1, cls_tok, W_q, W_k, W_v, W_t, W2):
    fold = _fold_weights(X, W1, cls_tok, W_q, W_k, W_v, W_t, W2)
    st = _build_stationary(fold["aw"], fold["dG"])
    per_core = _prep_planes(X)

    nc = _get_nc()
    idx_arr = np.zeros((128, 8), np.int16)
    idx_arr[:16] = (np.arange(8)[None, :] * 16 + np.arange(16)[:, None])
    ins = [{"xp1": np.concatenate([p[:, :2048], st], axis=1), "xp2": p[:, 2048:]}
           for p in per_core]
    for m in ins:
        m["xi"] = idx_arr
    res = run_bass_kernel_spmd(nc, ins, core_ids=list(range(NCORES)))
    loss = _host_finish([r["out"] for r in res.results], fold, y)
    return np.float32(loss)
